# revision 7
# baseline (speedup 1.0000x reference)
"""Distributed Trainium2 attention kernel (8 NeuronCores, head-sharded TP).

Reference computation (per problem spec, hardcoded shapes):
  B=2, S=2048, HID=2048, H=32 q-heads, HKV=8 kv-heads, D=64, GQA ratio 4.
  q/k/v projections -> RoPE(q,k) -> causal softmax attention -> out proj wo.

Sharding: tensor-parallel over heads; core i owns q-heads 4i..4i+3 and
kv-head i. Two AllToAlls convert head-sharding -> seq-sharding before the
row-split wo matmul. All on-device compute in transposed [feature, seq]
layout.

v2 design (single unified pipeline; v1 measured 438us, NTFF-trace-driven):
  - ONE software-pipelined attention stream over all 16 (hp, b, qc)
    blocks (emitted as two phase calls only for SBUF pool staging).
    Everything else -- projection groups g1..g7, the wo weight load, and
    the wo-even matmul tiles -- is woven into it as PE filler thunks, so
    the tensor engine never has a serial phase and the HAM clock stays
    released.
  - v1 served proj0 + proj-b1 serially/semi-serially before/inside the
    first attention scope: first exp landed at 91us.  Here attention
    emission starts right after g0+g1 (~35us).
  - A2A#1 (hp0) fires after block 7.  v1 fired it at 252us and the
    wo-even tiles woven behind it head-of-line-blocked the PE queue for
    34us (and stalled the last 48 attention tiles behind them).  With the
    earlier start + flat weaving, hp0 completes ~90us earlier and the ev
    weave begins only at global tile EV_START, past aot_a's worst-case
    landing time.
  - ACT queue hygiene: the exp stream (160 instructions, the attention
    pacer at (N+352)/1.2ns each) shares its queue only with the tiny
    h-load/wo-load DMA triggers.  PSUM->SBUF copies ride the DVE,
    swap/krope/cc/aot/out DMA triggers ride the sync HWDGE queue (gpsimd
    SWDGE is too slow/late for bulk loads).
  - Causal trimming of scores/exp/PV to the valid column range; scores
    head-pairs run concurrently on the PE via row tiling (krope halves at
    base partitions 0/64 -> tile_position (0,0)/(64,0)).
  - Divide tail: PV stationary is [ones | 31-pad | v] so the softmax
    denominator lands on PSUM partition 0 (the only base the approx-recip
    custom DVE op supports) and v-outputs land 32-bank aligned.  Both
    halves' reciprocals go in one [1,2,512] DVE call; the broadcast
    across partitions is a [1,128] fp16 ones-column matmul on the PE.
    Each block's divide is deferred into the next block so the PE never
    waits for it.
  - SBUF phases: hstream+ropetmp (proj pools, ~96KB) close at the phase
    seam once the proj thunks drain; only then does wo_sb (64KB) open.
    PSUM: psS (2x2 banks scores lookahead) + poa/pob + rbc + shared pf
    bank = 8.
  - Output written bf16 (host casts back to f32).
"""

import collections
import os
import sys

import numpy as np
import ml_dtypes

sys.path.insert(0, "/opt/trn_rl_repo")

import concourse.bass as bass  # noqa: E402
import concourse.mybir as mybir  # noqa: E402
import concourse.tile as tile  # noqa: E402
from concourse import bacc  # noqa: E402
from concourse.bass_utils import run_bass_kernel_spmd  # noqa: E402

F32 = mybir.dt.float32
BF16 = mybir.dt.bfloat16

H, HKV, D = 32, 8, 64
HID = 2048
B, S = 2, 2048
BS = B * S            # 4096 flattened (b, s)
NCORES = 8
NHQ = H // NCORES     # 4 local q heads
MQ = NHQ * D          # 256 local q rows
SCHUNK = BS // NCORES  # 512 output rows per core

LAST_EXEC_NS = None

# global tile index (0..159) at which wo-even weaving may start; must be
# past aot_a's worst-case landing (A2A#1 trigger + skew + flight + DMA).
EV_START = 118
PROJ_RATE = 4          # proj thunks per tile while they last
SEAM = 7               # blocks [0:SEAM) phase A, [SEAM:16) phase B


def _build(reps=1):
    nc = bacc.Bacc("TRN2", target_bir_lowering=False, debug=False,
                   num_devices=NCORES)

    hidden4 = nc.dram_tensor("hidden4", [8, 16, 128, 512], BF16,
                             kind="ExternalInput")
    wqT = nc.dram_tensor("wqT", [HID, MQ], BF16, kind="ExternalInput")
    wkvT = nc.dram_tensor("wkvT", [HID, 2 * D], BF16, kind="ExternalInput")
    woT = nc.dram_tensor("woT", [H * D, HID], BF16, kind="ExternalInput")
    cosT = nc.dram_tensor("cosT", [128, S], BF16, kind="ExternalInput")
    sinT = nc.dram_tensor("sinT", [128, S], BF16, kind="ExternalInput")
    maskb = nc.dram_tensor("maskb", [128, 2, 128], BF16, kind="ExternalInput")
    ident = nc.dram_tensor("ident", [128, D], BF16, kind="ExternalInput")
    out = nc.dram_tensor("out", [SCHUNK, HID], BF16, kind="ExternalOutput")

    from concourse.tile import add_dep_helper

    with tile.TileContext(nc) as tc:
        with (
            tc.tile_pool(name="persist", bufs=1) as persist,
            tc.tile_pool(name="qkv", bufs=1) as qkv,
            tc.tile_pool(name="dram", bufs=1, space="DRAM") as dram,
        ):
            # ---- persistent SBUF loads -------------------------------------
            wq_sb = persist.tile([128, 16, MQ], BF16)
            wqr = wqT.rearrange("(c p) m -> p c m", p=128)
            nc.sync.dma_start(wq_sb[:, 0:2, :], wqr[:, 0:2, :])
            nc.sync.dma_start(wq_sb[:, 2:16, :], wqr[:, 2:16, :])
            wkv_sb = persist.tile([128, 16, 2 * D], BF16)
            nc.scalar.dma_start(wkv_sb,
                                wkvT.rearrange("(c p) m -> p c m", p=128))
            cos_sb = persist.tile([128, S], BF16)
            cos_dma = nc.sync.dma_start(cos_sb, cosT[:, :])
            sin_sb = persist.tile([128, S], BF16)
            sin_dma = nc.sync.dma_start(sin_sb, sinT[:, :])
            mask_sb = persist.tile([128, 2, 128], BF16)
            nc.sync.dma_start(mask_sb, maskb[:, :, :])
            id_sb = persist.tile([128, D], BF16)
            nc.sync.dma_start(id_sb, ident[:, :])
            ones128 = persist.tile([1, 128], mybir.dt.float16)
            nc.vector.memset(ones128, 1.0)

            # ---- qkv outputs ----------------------------------------------
            qrope = qkv.tile([128, 2, BS], BF16)     # [64*(h%2), h//2, b*S+s]
            krope = qkv.tile([128, BS], BF16)        # duplicated both halves
            vT_sb = qkv.tile([128, BS], BF16)        # rows 64:128 used
            # per k-tile 96-wide stationary: col 0 = ones (denominator on
            # PSUM partition 0, where the approx-recip custom op works),
            # cols 1:32 zero pad (v outputs land 32-aligned for the DVE),
            # cols 32:96 = v
            v_sb = qkv.tile([128, B, 16 * 128], BF16)
            nc.vector.memset(v_sb, 0.0)
            for b in range(B):
                for kt in range(16):
                    nc.vector.memset(v_sb[:, b, kt * 128: kt * 128 + 1],
                                     1.0)

            cc_in_a = dram.tile([NCORES, 128, 512], BF16)  # heads 0,1
            cc_in_b = dram.tile([NCORES, 128, 512], BF16)  # heads 2,3
            cc_out_a = dram.tile([NCORES * 128, 512], BF16)
            cc_out_b = dram.tile([NCORES * 128, 512], BF16)

            for rep in range(reps):
                h_dmas = []
                P = {}            # live pool holder (phase A / phase B)
                pstate = {}
                pending_div = []

                def flush_div():
                    # divide tail of the previous block: reciprocal of the
                    # denominator row (both halves in one [1,2,512] DVE
                    # call), PE broadcast across partitions, multiply, ship
                    # to the cc buffer.
                    if not pending_div:
                        return
                    divp = P["divp"]
                    osb, cc_tile, slot = pending_div.pop()
                    recip = divp.tile([1, 2, 512], mybir.dt.float16,
                                      tag="recip", bufs=1, name="recip")
                    rec32 = divp.tile([1, 2, 512], F32, tag="rec32",
                                      bufs=1, name="rec32")
                    with nc.allow_low_precision(
                            reason="fp16 recip feeds the fp16 ones-matmul "
                                   "broadcast; 10-bit mantissa is ample"):
                        nc.vector.reciprocal_approx_fast(
                            rec32, osb[0:1, :, :])
                        nc.vector.tensor_copy(recip, rec32)
                    for half in range(2):
                        rbc = P["psF"].tile([128, 512], F32, tag="rbc",
                                            name=f"rbc_{half}")
                        nc.tensor.matmul(
                            rbc,
                            ones128[0:1, :],
                            recip[0:1, half, :],
                            start=True, stop=True)
                        ao = divp.tile([128, 512], BF16, tag=f"ao{half}",
                                       name=f"ao_{half}", bufs=2)
                        nc.vector.tensor_mul(ao[64:128, :],
                                             osb[64:128, half, :],
                                             rbc[64:128, :])
                        nc.sync.dma_start(
                            cc_tile[slot, bass.ts(half, 64), :],
                            ao[64:128, :])

                def attn_stream(psS, psO, blocks, base, filler=None,
                                after_block=None):
                    """Software-pipelined attention stream.

                    blocks: list of (hp, b, qc). Scores are emitted with a
                    2-tile lookahead across block boundaries so the PE never
                    head-of-line blocks on the exp of a block's last tiles.
                    filler(gidx) is called once per tile with the global
                    tile index (base + local); it emits a slab of
                    independent PE work to keep the tensor engine dense
                    while the ACT-bound exp stream paces attention.
                    after_block: local block index -> callback.
                    """
                    attnp, divp = P["attnp"], P["divp"]
                    seq = []
                    for bi, (hp, b, qc) in enumerate(blocks):
                        for kt in range(4 * qc + 4):
                            seq.append((bi, hp, b, qc, kt))
                    pss = {}

                    def emit_scores(i):
                        bi, hp, b, qc, kt = seq[i]
                        j = kt - 4 * qc
                        lo = max(j, 0) * 128
                        kcols = bass.ds(b * S + kt * 128, 128)
                        qc2 = bass.ds(b * S + qc * 512 + lo, 512 - lo)
                        ps_s = psS.tile([128, 2, 512], F32, tag="pss",
                                        name=f"ps_s_{hp}_{b}_{qc}_{kt}")
                        nc.tensor.matmul(ps_s[:, 0, lo:],
                                         krope[0:64, kcols],
                                         qrope[0:64, hp, qc2],
                                         start=True, stop=True)
                        nc.tensor.matmul(ps_s[:, 1, lo:],
                                         krope[64:128, kcols],
                                         qrope[64:128, hp, qc2],
                                         start=True, stop=True)
                        pss[i] = ps_s

                    LOOK = 2
                    for i in range(min(LOOK, len(seq))):
                        emit_scores(i)
                    pso = {}
                    for i, (bi, hp, b, qc, kt) in enumerate(seq):
                        if i + LOOK < len(seq):
                            emit_scores(i + LOOK)
                        nkt = 4 * qc + 4
                        if kt == 0:
                            # previous block's divide tail lands here, after
                            # this block's first scores are already in flight
                            flush_div()
                            pso[bi] = (psO.tile([128, 512], F32, tag="poa",
                                                name=f"poa_{base}_{bi}"),
                                       psO.tile([128, 512], F32, tag="pob",
                                                name=f"pob_{base}_{bi}"))
                        ps_oa, ps_ob = pso[bi]
                        ps_s = pss.pop(i)
                        j = kt - 4 * qc
                        lo = max(j, 0) * 128
                        attn = attnp.tile([128, 2, 512], BF16, tag="attn")
                        nc.scalar.activation(
                            attn[:, :, lo:], ps_s[:, :, lo:],
                            mybir.ActivationFunctionType.Exp,
                            scale=0.125)
                        if j >= 0:
                            nc.vector.tensor_mul(
                                attn[:, :, lo:lo + 128],
                                attn[:, :, lo:lo + 128], mask_sb)
                        vs = v_sb[:, b, kt * 128: kt * 128 + 128]
                        nc.tensor.matmul(ps_oa[:, lo:], vs, attn[:, 0, lo:],
                                         start=(kt == 0),
                                         stop=(kt == nkt - 1),
                                         skip_group_check=True)
                        nc.tensor.matmul(ps_ob[:, lo:], vs, attn[:, 1, lo:],
                                         start=(kt == 0),
                                         stop=(kt == nkt - 1),
                                         skip_group_check=True)
                        if filler is not None:
                            filler(base + i)
                        if kt == nkt - 1:
                            cc_tile = cc_in_a if hp == 0 else cc_in_b
                            osb = divp.tile([128, 2, 512], F32, tag="osb",
                                            name=f"osb_{base}_{bi}")
                            nc.vector.tensor_copy(osb[:, 0, :], ps_oa)
                            nc.vector.tensor_copy(osb[:, 1, :], ps_ob)
                            pending_div.append((osb, cc_tile, b * 4 + qc))
                            if after_block and bi in after_block:
                                after_block[bi]()

                blocks = [(hp, b, qc) for hp in (0, 1) for b in (0, 1)
                          for qc in range(4)]

                # ================= emission ================================
                with (
                    tc.tile_pool(name="attnp", bufs=8) as attnp,
                    tc.tile_pool(name="divp", bufs=3) as divp,
                ):
                    P["attnp"] = attnp
                    P["divp"] = divp

                    # ======== phase A: blocks 0..SEAM-1 + proj weaving ======
                    ctx_sa = tc.tile_pool(name="psSa", bufs=2, space="PSUM")
                    ctx_oa = tc.tile_pool(name="psOa", bufs=1, space="PSUM")
                    ctx_fa = tc.tile_pool(name="psFa", bufs=1, space="PSUM")
                    psS_a = ctx_sa.__enter__()
                    psO_a = ctx_oa.__enter__()
                    psF_a = ctx_fa.__enter__()
                    P["psF"] = psF_a
                    # hstream bufs=3: with 2, a t_dma emitted before the
                    # previous-but-one group's readers would race its buffer
                    # (WAR deps only cover already-emitted readers).
                    ctx_h = tc.tile_pool(name="hstream", bufs=3)
                    hstream = ctx_h.__enter__()
                    ctx_r = tc.tile_pool(name="ropetmp", bufs=2)
                    ropetmp = ctx_r.__enter__()

                    # ---- projection thunks (group g: 512 seq cols of
                    # batch g//4) ------------------------------------------
                    def t_dma(g, split=1):
                        def t():
                            h_sb = hstream.tile([128, 16, 512], BF16,
                                                tag="h", name=f"h_sb_{g}")
                            pstate[("h", g)] = h_sb
                            if split == 3:
                                nc.scalar.dma_start(
                                    h_sb[:, 0:4, :], hidden4[g, 0:4, :, :]
                                    .rearrange("c p n -> p c n"))
                                nc.scalar.dma_start(
                                    h_sb[:, 4:8, :], hidden4[g, 4:8, :, :]
                                    .rearrange("c p n -> p c n"))
                                h_dmas.append(nc.scalar.dma_start(
                                    h_sb[:, 8:16, :], hidden4[g, 8:16, :, :]
                                    .rearrange("c p n -> p c n")))
                            elif split == 2:
                                nc.scalar.dma_start(
                                    h_sb[:, 0:8, :], hidden4[g, 0:8, :, :]
                                    .rearrange("c p n -> p c n"))
                                h_dmas.append(nc.scalar.dma_start(
                                    h_sb[:, 8:16, :], hidden4[g, 8:16, :, :]
                                    .rearrange("c p n -> p c n")))
                            else:
                                h_dmas.append(nc.scalar.dma_start(
                                    h_sb, hidden4[g, :, :, :]
                                    .rearrange("c p n -> p c n")))
                        return t

                    def t_mm(g, tgt, cp):
                        def t():
                            if cp == 0:
                                pstate[("pf", g, tgt)] = P["psF"].tile(
                                    [128, 512], F32, tag="pf",
                                    name=f"pf_{g}_{tgt}")
                            ps = pstate[("pf", g, tgt)]
                            h_sb = pstate[("h", g)]
                            for c in (2 * cp, 2 * cp + 1):
                                if tgt == 0:
                                    w = wq_sb[:, c, 0:128]
                                elif tgt == 1:
                                    w = wq_sb[:, c, 128:256]
                                else:
                                    w = wkv_sb[:, c, :]
                                nc.tensor.matmul(
                                    ps, w, h_sb[:, c, :],
                                    start=(c == 0), stop=(c == 15),
                                    skip_group_check=True)
                        return t

                    def t_copy(g, tgt):
                        def t():
                            if tgt == 0:
                                pstate[("x", g)] = ropetmp.tile(
                                    [128, 3, 512], BF16, tag="x",
                                    name=f"x_{g}")
                            x = pstate[("x", g)]
                            ps = pstate[("pf", g, tgt)]
                            cols = bass.ds(g * 512, 512)
                            if tgt < 2:
                                nc.vector.tensor_copy(x[:, tgt, :], ps)
                            else:
                                nc.vector.tensor_copy(x[0:64, 2, :],
                                                      ps[0:64, :])
                                nc.vector.tensor_copy(vT_sb[64:128, cols],
                                                      ps[64:128, :])
                        return t

                    def t_rope(g, part):
                        def t():
                            cols = bass.ds(g * 512, 512)
                            x = pstate[("x", g)]
                            if part == 0:
                                swap = ropetmp.tile([128, 3, 512], BF16,
                                                    tag="swap",
                                                    name=f"swap_{g}")
                                pstate[("swap", g)] = swap
                                for blk in range(2):
                                    p0 = blk * 64
                                    nc.sync.dma_start(
                                        swap[p0: p0 + 32, :, :],
                                        x[p0 + 32: p0 + 64, :, :])
                                    nc.sync.dma_start(
                                        swap[p0 + 32: p0 + 64, :, :],
                                        x[p0: p0 + 32, :, :])
                            elif part == 1:
                                swap = pstate[("swap", g)]
                                tmp = ropetmp.tile([128, 3, 512], BF16,
                                                   tag="tmp",
                                                   name=f"tmp_{g}")
                                pstate[("tmp", g)] = tmp
                                scols = bass.ds((g % 4) * 512, 512)
                                for tt, rows in ((0, 128), (1, 128),
                                                 (2, 64)):
                                    nc.vector.tensor_mul(
                                        tmp[:rows, tt, :],
                                        swap[:rows, tt, :],
                                        sin_sb[:rows, scols])
                            else:
                                tmp = pstate[("tmp", g)]
                                cosp = ropetmp.tile([128, 3, 512], BF16,
                                                    tag="cosp",
                                                    name=f"cosp_{g}")
                                scols = bass.ds((g % 4) * 512, 512)
                                for tt, rows in ((0, 128), (1, 128),
                                                 (2, 64)):
                                    nc.vector.tensor_mul(
                                        cosp[:rows, tt, :],
                                        x[:rows, tt, :],
                                        cos_sb[:rows, scols])
                                nc.vector.tensor_add(
                                    qrope[:, 0, cols], tmp[:, 0, :],
                                    cosp[:, 0, :])
                                nc.vector.tensor_add(
                                    qrope[:, 1, cols], tmp[:, 1, :],
                                    cosp[:, 1, :])
                                nc.vector.tensor_add(
                                    krope[0:64, cols], tmp[0:64, 2, :],
                                    cosp[0:64, 2, :])
                                nc.sync.dma_start(krope[64:128, cols],
                                                  krope[0:64, cols])
                        return t

                    def t_tr(g, j):
                        def t():
                            b = g // 4
                            st = g * 512 + j * 128
                            kt = (st - b * S) // 128
                            tp = P["psF"].tile([128, D], BF16, tag="pf",
                                               name=f"tp_{g}_{j}")
                            nc.tensor.transpose(
                                tp, vT_sb[64:128, bass.ds(st, 128)],
                                id_sb[64:128, :])
                            nc.vector.tensor_copy(
                                v_sb[:, b, kt * 128 + 64: kt * 128 + 128],
                                tp)
                        return t

                    def group_thunks(g):
                        th = []
                        for tgt in range(3):
                            for cp in range(8):
                                th.append(t_mm(g, tgt, cp))
                            th.append(t_copy(g, tgt))
                        for part in range(3):
                            th.append(t_rope(g, part))
                        for j in range(4):
                            th.append(t_tr(g, j))
                        return th

                    # ---- prologue: g0 + g1 serial --------------------------
                    t_dma(0, split=3)()
                    t_dma(1, split=2)()
                    t_dma(2)()
                    if rep == 0:
                        add_dep_helper(cos_dma.ins, h_dmas[0].ins, sync=True,
                                       reason="cos table after first hidden")
                        add_dep_helper(sin_dma.ins, h_dmas[1].ins, sync=True,
                                       reason="sin table after second hidden")
                    for g in (0, 1):
                        for th in group_thunks(g):
                            th()
                    t_dma(3)()

                    # ---- thunk queue: g2..g7 (+h loads two groups ahead) ---
                    thunks = collections.deque()
                    for g in range(2, 8):
                        if g + 2 < 8:
                            thunks.append(t_dma(g + 2))
                        thunks.extend(group_thunks(g))

                    def filler_a(gidx):
                        for _ in range(PROJ_RATE):
                            if thunks:
                                thunks.popleft()()

                    attn_stream(psS_a, psO_a, blocks[:SEAM], base=0,
                                filler=filler_a)
                    while thunks:
                        thunks.popleft()()

                    # ======== seam: swap proj pools for the wo pool =========
                    ctx_r.__exit__(None, None, None)
                    ctx_h.__exit__(None, None, None)
                    ctx_fa.__exit__(None, None, None)
                    ctx_oa.__exit__(None, None, None)
                    ctx_sa.__exit__(None, None, None)

                    ctx_w = tc.tile_pool(name="wop", bufs=1)
                    wop = ctx_w.__enter__()
                    wo_sb = wop.tile([128, 16, HID], BF16)
                    ev_sb = wop.tile([128, 4, HID], BF16)
                    aot_a = wop.tile([128, 8, 512], BF16)
                    aot_b = wop.tile([128, 8, 512], BF16)
                    for ch in range(4):
                        nc.scalar.dma_start(
                            wo_sb[:, bass.ts(ch, 4), :],
                            woT.rearrange("(c p) n -> p c n",
                                          p=128)[:, bass.ts(ch, 4), :])

                    # ---- wo-even thunks (consume aot_a) --------------------
                    estate = {}

                    def t_ev(st, nh, ii):
                        def t():
                            ns = bass.ts(nh, 512)
                            if ii == 0:
                                estate[(st, nh)] = P["psF"].tile(
                                    [128, 512], F32, tag="pf",
                                    name=f"ps_e_{st}_{nh}")
                            ps_e = estate[(st, nh)]
                            for k in (2 * ii, 2 * ii + 1):
                                nc.tensor.matmul(
                                    ps_e, aot_a[:, k, bass.ts(st, 128)],
                                    wo_sb[:, 2 * k, ns],
                                    start=(k == 0), stop=(k == 7),
                                    skip_group_check=True)
                        return t

                    def t_evc(st, nh):
                        def t():
                            ns = bass.ts(nh, 512)
                            nc.vector.tensor_copy(ev_sb[:, st, ns],
                                                  estate[(st, nh)])
                        return t

                    evq = collections.deque()
                    for st in range(4):
                        for nh in range(4):
                            for ii in range(4):
                                evq.append(t_ev(st, nh, ii))
                            evq.append(t_evc(st, nh))

                    def fire_a2a1():
                        flush_div()
                        nc.gpsimd.collective_compute(
                            "AllToAll", mybir.AluOpType.bypass,
                            replica_groups=[list(range(NCORES))],
                            ins=[cc_in_a.opt()],
                            outs=[cc_out_a.opt()])
                        nc.sync.dma_start(
                            aot_a,
                            cc_out_a.rearrange("(c p) n -> p c n", p=128))

                    def filler_b(gidx):
                        if gidx >= EV_START and evq:
                            evq.popleft()()
                            if evq and gidx >= EV_START + 20:
                                evq.popleft()()

                    # ======== phase B: blocks SEAM..15 + ev weaving =========
                    with (
                        tc.tile_pool(name="psSb", bufs=2,
                                     space="PSUM") as psS_b,
                        tc.tile_pool(name="psOb", bufs=1,
                                     space="PSUM") as psO_b,
                        tc.tile_pool(name="psFb", bufs=1,
                                     space="PSUM") as psF_b,
                    ):
                        P["psF"] = psF_b
                        base = sum(4 * qc + 4 for _, _, qc in blocks[:SEAM])
                        attn_stream(psS_b, psO_b, blocks[SEAM:], base=base,
                                    filler=filler_b,
                                    after_block={7 - SEAM: fire_a2a1})
                        flush_div()

                        # ---- A2A #2 (hp1) + drain --------------------------
                        nc.gpsimd.collective_compute(
                            "AllToAll", mybir.AluOpType.bypass,
                            replica_groups=[list(range(NCORES))],
                            ins=[cc_in_b.opt()],
                            outs=[cc_out_b.opt()])
                        ccob = cc_out_b.rearrange("(c p) n -> p c n", p=128)
                        nc.sync.dma_start(aot_b[:, 0:2, :], ccob[:, 0:2, :])
                        nc.sync.dma_start(aot_b[:, 2:8, :], ccob[:, 2:8, :])
                        while evq:
                            evq.popleft()()

                    # ============= wo odd chunks + merge ====================
                    with (
                        tc.tile_pool(name="psWo", bufs=2,
                                     space="PSUM") as psWo,
                        tc.tile_pool(name="outp", bufs=2) as outp,
                    ):
                        for st in range(4):
                            ps_w = psWo.tile([128, HID], F32, tag="psw",
                                             name=f"ps_o_{st}")
                            for i in range(8):
                                for nh in range(4):
                                    ns = bass.ts(nh, 512)
                                    nc.tensor.matmul(
                                        ps_w[:, ns],
                                        aot_b[:, i, bass.ts(st, 128)],
                                        wo_sb[:, 2 * i + 1, ns],
                                        start=(i == 0), stop=(i == 7))
                            osb = outp.tile([128, HID], BF16, tag="osb")
                            # half-granularity merge+store: the first DMA
                            # streams while the second half merges
                            for hh in range(2):
                                hs = bass.ts(hh, HID // 2)
                                nc.vector.tensor_add(osb[:, hs],
                                                     ps_w[:, hs],
                                                     ev_sb[:, st, hs])
                                nc.sync.dma_start(
                                    out[bass.ts(st, 128), hs], osb[:, hs])
                    ctx_w.__exit__(None, None, None)

    nc.compile()
    return nc


_NC_CACHE = {}


def _get_nc(reps=1):
    key = f"nc{reps}"
    if key not in _NC_CACHE:
        _NC_CACHE[key] = _build(reps)
    return _NC_CACHE[key]


def _prep_inputs(hidden_states, cos, sin, wq, wk, wv, wo):
    bf = ml_dtypes.bfloat16
    hiddenT = np.ascontiguousarray(
        hidden_states.reshape(BS, HID).T).astype(bf)       # [HID, BS]
    hidden4 = np.ascontiguousarray(
        hiddenT.reshape(16, 128, 8, 512).transpose(2, 0, 1, 3))
    woT = np.ascontiguousarray(np.asarray(wo).T).astype(bf)

    cos2 = np.asarray(cos)[:, 0, :]          # [S, D]
    sin2 = np.asarray(sin)[:, 0, :]
    cosTb = cos2.T                            # [D, S]
    sinTb = sin2.T
    sin_signed = np.concatenate([-sinTb[:32], sinTb[32:]], axis=0)
    cos_full = np.tile(cosTb, (2, 1)).astype(bf)       # [128, S]
    sin_full = np.tile(sin_signed, (2, 1)).astype(bf)  # [128, S]

    # triangular causal band mask, duplicated for the two heads of a pair
    kk = np.arange(128)[:, None]
    qq = np.arange(128)[None, :]
    maskb1 = np.where(kk > qq, 0.0, 1.0).astype(np.float32).astype(bf)
    maskb = np.ascontiguousarray(
        np.broadcast_to(maskb1[:, None, :], (128, 2, 128)))

    ident_np = np.zeros((128, D), np.float32)
    ident_np[64:128, :] = np.eye(D)
    ident_np = ident_np.astype(bf)

    wq = np.asarray(wq)
    wk = np.asarray(wk)
    wv = np.asarray(wv)
    in_maps = []
    for i in range(NCORES):
        wq_i = wq[i * MQ:(i + 1) * MQ, :]                      # [256, HID]
        wkv_i = np.concatenate([wk[i * D:(i + 1) * D, :],
                                wv[i * D:(i + 1) * D, :]], axis=0)
        in_maps.append({
            "hidden4": hidden4,
            "wqT": np.ascontiguousarray(wq_i.T).astype(bf),
            "wkvT": np.ascontiguousarray(wkv_i.T).astype(bf),
            "woT": woT,
            "cosT": cos_full,
            "sinT": sin_full,
            "maskb": maskb,
            "ident": ident_np,
        })
    return in_maps


def kernel(hidden_states, cos, sin, wq, wk, wv, wo):
    global LAST_EXEC_NS
    reps = int(os.environ.get("KREPS", "1"))
    in_maps = _prep_inputs(np.asarray(hidden_states, np.float32),
                           cos, sin, wq, wk, wv, wo)
    nc = _get_nc(reps)
    res = run_bass_kernel_spmd(nc, in_maps, core_ids=list(range(NCORES)),
                               trace=bool(int(os.environ.get("BASS_TRACE",
                                                             "0"))))
    LAST_EXEC_NS = res.exec_time_ns
    outs = [res.results[i]["out"].astype(np.float32) for i in range(NCORES)]
    full = np.concatenate(outs, axis=0).reshape(B, S, HID)
    return full
